# revision 3
# baseline (speedup 1.0000x reference)
"""Trainium2 Bass kernel for nn_Interpolator (quadratic-form kernel interpolation).

Math (T=8192 targets, C=8192 contexts, D=64, DY=32):
    S = W + W^T
    scores[t,c] = (z_t - z_c)^T W (z_t - z_c)
                = q_tt[t] + q_cc[c] - z_t^T S z_c
    theta = exp(-scores);  out = (theta @ y_context) / theta.sum(-1, keepdim)

q_tt[t] scales whole theta rows and cancels in the normalization -> dropped.
q_cc[c] = 0.5 * z_c^T S z_c is folded into the main matmul contraction:
the stationary operand LC has 128 rows: rows 0..63 = zc^T, rows 64..127 =
0.5*(zc .* (S zc)); the moving operand RT has rows 0..63 = S^T z_t and
rows 64..127 = -1. A single K=128 fp16 matmul then yields cross - q_cc
directly (matmul cost depends only on moving columns, so the fold is free).

Sharding: data-parallel over targets; each of the 8 cores takes T/8 = 1024
targets and the full context set.

Per-core device program, v2 — the ACT engine's exp stream is the roofline
(8.4M exps/core at 1 elem/lane/cycle @ 1.2 GHz + ~171 cyc/instruction), so
chunks are processed in groups of 3 with a 1024-wide + a 2048-wide
ACTIVATE per group (fewer instruction overheads than 64x 1024-wide):
  - PSUM: PB [128,1024] (2 banks) holds chunk 3g; PA [128,2048] (4 banks)
    holds chunks 3g+1, 3g+2; o2a/o2b (1 bank each) accumulate the output.
    Chunk 63 is a final lone PB tile whose exp is split 2x512 so the
    epilogue starts earlier.
  - Per-group emission order keeps both engines ~95% busy with no FIFO
    head-of-line stalls: exp(B_g) | A(g) matmuls | exp(A_g) | zs piece |
    B(g+1) matmuls | mm2(A_{g-1}) | mm2(B_g).
  - mm2: o2 [33,1024] accumulates YA^T @ theta; y_aug col 32 = ones gives
    the denominator row.
  - input DMAs split across four HWDGE queues (sync: zt + zc block 0;
    vector/tensor: zc bulk halves; scalar: W + y) so the W -> S -> RT/zs
    chain and chunk 0 start ~3us earlier and zc streams at 2x queue rate.
  - the 16 zs pieces (LC rows 64..127): pieces 0-1 in the prelude (direct
    DVE write), the rest paced one-per-group borrowing PB right after its
    exp, staged via SBUF and landed by a subtile-tracked SBUF->SBUF DMA.
  - epilogue: mm2(B21 h0) stops o2a early; ACT copies o2a while DVE
    copies o2b, each half DMA'd on its own queue.
Host: shard/transpose/cast inputs (layout only), concat per-core [33,1024]
outputs, divide numerator rows by the denominator row.
"""

import ml_dtypes
import numpy as np

import concourse.bacc as bacc
import concourse.bass as bass
import concourse.mybir as mybir
import concourse.tile as tile
from concourse.bass_utils import run_bass_kernel_spmd

F32 = mybir.dt.float32
F16 = mybir.dt.float16
BF16 = mybir.dt.bfloat16

T, C, D, DY = 8192, 8192, 64, 32
NCORES = 8
TL = T // NCORES          # 1024 targets per core
NCHUNK = C // 128         # 64 context chunks of 128
NPIECE = C // 512         # 16 zs pieces of 512 contexts
NGROUP = 21               # full groups: B=3g, A=(3g+1, 3g+2); chunk 63 lone B
NWARM = 4


def _build_kernel_body(tc: tile.TileContext):
    nc = tc.nc
    Exp = mybir.ActivationFunctionType.Exp

    wwt_d = nc.dram_tensor("wwt", [D, 2 * D], F32, kind="ExternalInput")
    zt_d = nc.dram_tensor("ztt", [D, TL], F16, kind="ExternalInput")
    zc0_d = nc.dram_tensor("zcb0", [D, 1024], F16, kind="ExternalInput")
    zcv_d = nc.dram_tensor("zcbv", [D, 3584], F16, kind="ExternalInput")
    zct_d = nc.dram_tensor("zcbt", [D, 3584], F16, kind="ExternalInput")
    y_d = nc.dram_tensor("yck", [128, NCHUNK * DY], BF16, kind="ExternalInput")
    out_d = nc.dram_tensor("out", [DY + 1, TL], F32, kind="ExternalOutput")

    with (
        tc.tile_pool(name="sb", bufs=1) as sb,
        tc.tile_pool(name="pp", bufs=1, space="PSUM") as pp,
    ):
        # ---- resident SBUF slabs ----
        LC = sb.tile([128, C], F16, name="lc")
        RT = sb.tile([128, TL], F16, name="rt")
        ZT = sb.tile([D, TL], F16, name="zt")
        YT = sb.tile([128, NCHUNK * DY], BF16, name="yt")
        YA = sb.tile([128, NCHUNK, DY + 1], BF16, name="ya")
        WW = sb.tile([D, 2 * D], F32, name="ww")
        SS = sb.tile([D, D], F16, name="ss")
        SSH = sb.tile([D, D], F16, name="ssh")
        THA0 = sb.tile([128, 2048], BF16, name="tha0")
        THA1 = sb.tile([128, 2048], BF16, name="tha1")
        THB0 = sb.tile([128, TL], BF16, name="thb0")
        THB1 = sb.tile([128, TL], BF16, name="thb1")
        OSB = sb.tile([DY + 1, TL], F32, name="osb")
        WRM = sb.tile([128, 512], BF16, name="wrm")
        EXD = sb.tile([D, 1], F32, name="exd")
        LCS0 = sb.tile([D, 512], F16, name="lcs0")
        LCS1 = sb.tile([D, 512], F16, name="lcs1")
        LCS = [LCS0, LCS1]
        THA = [THA0, THA1]
        THB = [THB0, THB1]

        # ---- PSUM: PA 4 banks (A pair), PB 2 banks (B chunk), o2 2 banks
        PA = pp.tile([128, 2048], F32, tag="pa", name="pa")
        PB = pp.tile([128, TL], F32, tag="pb", name="pb")
        o2a = pp.tile([DY + 1, 512], F32, tag="o2a", name="o2a")
        o2b = pp.tile([DY + 1, 512], F32, tag="o2b", name="o2b")
        O2 = [o2a, o2b]

        # ---- input DMAs: four queues so the critical chain (W -> SS ->
        # RT/zs -> chunk 0) and the zc bulk stream in parallel ----
        nc.scalar.dma_start(out=WW, in_=wwt_d.ap())
        nc.sync.dma_start(out=ZT, in_=zt_d.ap())
        nc.sync.dma_start(out=LC[:D, 0:1024], in_=zc0_d.ap())
        nc.gpsimd.dma_start(out=LC[:D, 1024:4608], in_=zcv_d.ap())
        nc.gpsimd.dma_start(out=LC[:D, 4608:8192], in_=zct_d.ap())
        half_y = NCHUNK * DY // 2
        nc.scalar.dma_start(out=YT[:, :half_y], in_=y_d.ap()[:, :half_y])

        # exp-table preload (so the first chunk ACTIVATE is cheap)
        nc.vector.memset(EXD, 0.0)
        nc.scalar.activation(EXD, EXD, Exp)
        nc.scalar.dma_start(out=YT[:, half_y:], in_=y_d.ap()[:, half_y:])

        # ---- PE warm-up burst into PA: flips HAM to 8/8 early ----
        nc.vector.memset(WRM, 0.5)
        for i in range(NWARM):
            nc.tensor.matmul(
                PA[:, i * 512 : (i + 1) * 512], WRM[:, 0:128], WRM,
                start=True, stop=True,
            )

        # ---- DVE prelude chain ----
        nc.vector.memset(RT[D:128, :], -1.0)
        nc.vector.tensor_add(SS, WW[:, 0:D], WW[:, D : 2 * D])   # fp16 S
        nc.vector.tensor_scalar_mul(SSH, SS, 0.5)                # fp16 S/2

        # ---- RT rows 0..63 = S^T zt, staged in PA banks 0-1; cast h0 on
        # ACT (idle) and h1 on DVE in parallel ----
        for h in range(2):
            sl = slice(h * 512, (h + 1) * 512)
            nc.tensor.matmul(PA[:D, sl], SS, ZT[:, sl], start=True, stop=True)
        nc.scalar.copy(RT[:D, 0:512], PA[:D, 0:512])
        nc.vector.tensor_copy(RT[:D, 512:1024], PA[:D, 512:1024])

        # ---- prelude zs pieces 0,1 (contexts 0:1024) staged in PA banks
        # 2-3, DVE writes LC rows 64:128 directly ----
        for k in range(2):
            sl = slice(512 * k, 512 * (k + 1))
            st = slice(1024 + 512 * k, 1024 + 512 * (k + 1))
            nc.tensor.matmul(PA[:D, st], SSH, LC[:D, sl], start=True, stop=True)
            nc.vector.tensor_mul(LC[D:128, sl], PA[:D, st], LC[:D, sl])

        # in-loop zs piece: borrow PB[:D, 0:512] right after exp(B_g);
        # the DVE mul lands in SBUF staging and an SBUF->SBUF DMA
        # (subtile-tracked) carries it into LC rows 64..127 so loop
        # matmuls wait only on the DMA.
        def zs_piece(k):
            sl = slice(512 * k, 512 * (k + 1))
            nc.tensor.matmul(PB[:D, 0:512], SSH, LC[:D, sl], start=True,
                             stop=True)
            nc.vector.tensor_mul(LCS[k % 2][:, 0:512], PB[:D, 0:512],
                                 LC[:D, sl])
            nc.sync.dma_start(out=LC[D:128, sl], in_=LCS[k % 2][:, 0:512])

        # y_aug piece q (16 chunks): [128, 16, 33]; col 32 = 1.0
        nc.vector.memset(YA[:, :, DY : DY + 1], 1.0)
        qy = NCHUNK // 4 * DY

        def ya_piece(q):
            nc.vector.tensor_copy(
                YA[:, q * 16 : (q + 1) * 16, 0:DY],
                YT[:, q * qy : (q + 1) * qy].rearrange("p (j d) -> p j d", d=DY),
            )

        ya_piece(0)

        # piece k >= 2 emitted at group g_k (ready well before chunk 4k)
        zs_at = {}
        for k in range(2, NPIECE):
            g_k = max(0, -(-4 * k // 3) - 2)
            while g_k in zs_at:
                g_k += 1
            zs_at[g_k] = k

        def score_mms(P, base, c):
            """two N=512 matmuls: scores of chunk c into P[:, base:base+1024]"""
            lhsT = LC[:, c * 128 : (c + 1) * 128]
            for h in range(2):
                nc.tensor.matmul(
                    P[:, base + h * 512 : base + (h + 1) * 512],
                    lhsT,
                    RT[:, h * 512 : (h + 1) * 512],
                    start=True, stop=True,
                )

        first_mm2 = [True, True]

        def mm2(th, base, c, stop=False):
            """o2 += YA[c]^T @ theta (both target halves)"""
            for h in range(2):
                nc.tensor.matmul(
                    O2[h][:, :],
                    YA[:, c, :],
                    th[:, base + h * 512 : base + (h + 1) * 512],
                    start=first_mm2[h],
                    stop=stop,
                )
                first_mm2[h] = False

        # ---- prologue of the software pipeline ----
        score_mms(PB, 0, 0)                      # B(0)

        # ---- main loop over 21 full groups ----
        for g in range(NGROUP):
            cB, cA1, cA2 = 3 * g, 3 * g + 1, 3 * g + 2
            nc.scalar.activation(THB[g % 2], PB, Exp)          # exp(B_g)
            score_mms(PA, 0, cA1)                              # A(g)
            score_mms(PA, 1024, cA2)
            nc.scalar.activation(THA[g % 2], PA, Exp)          # exp(A_g)
            if g in zs_at:
                zs_piece(zs_at[g])
            if g < NGROUP - 1:
                score_mms(PB, 0, cB + 3)                       # B(g+1)
            else:
                score_mms(PB, 0, 63)                           # final lone B
            if g > 0:
                mm2(THA[(g - 1) % 2], 0, cA1 - 3)              # mm2(A_{g-1})
                mm2(THA[(g - 1) % 2], 1024, cA2 - 3)
            mm2(THB[g % 2], 0, cB)                             # mm2(B_g)
            if g in (3, 8, 13):
                ya_piece({3: 1, 8: 2, 13: 3}[g])

        # ---- epilogue: final chunk 63 in PB, exp split 2x512 so the h0
        # accumulator closes and evacuates earlier ----
        g = NGROUP
        for h in range(2):
            sl = slice(h * 512, (h + 1) * 512)
            nc.scalar.activation(THB[g % 2][:, sl], PB[:, sl], Exp)
        mm2(THA[(g - 1) % 2], 0, 61)                           # mm2(A_20)
        mm2(THA[(g - 1) % 2], 1024, 62)
        mm2(THB[g % 2], 0, 63, stop=True)
        nc.scalar.copy(OSB[:, 0:512], o2a)
        nc.scalar.dma_start(out=out_d.ap()[:, 0:512], in_=OSB[:, 0:512])
        nc.vector.tensor_copy(OSB[:, 512:1024], o2b)
        nc.sync.dma_start(out=out_d.ap()[:, 512:1024], in_=OSB[:, 512:1024])


_CACHED = None


def _get_nc():
    global _CACHED
    if _CACHED is None:
        nc = bacc.Bacc(
            "TRN2",
            target_bir_lowering=False,
            debug=False,
            enable_asserts=False,
        )
        with tile.TileContext(nc) as tc:
            _build_kernel_body(tc)
        nc.compile()
        _CACHED = nc
    return _CACHED


def make_in_maps(z_context, y_context, z_target, W):
    """Host-side layout prep (transpose/reshape/cast only) + sharding."""
    z_context = np.asarray(z_context, dtype=np.float32)
    y_context = np.asarray(y_context, dtype=np.float32)
    z_target = np.asarray(z_target, dtype=np.float32)
    W = np.asarray(W, dtype=np.float32)

    zcT = np.ascontiguousarray(z_context.T.astype(np.float16))  # [64, 8192]
    zc0 = np.ascontiguousarray(zcT[:, 0:1024])
    zcv = np.ascontiguousarray(zcT[:, 1024:4608])
    zct = np.ascontiguousarray(zcT[:, 4608:8192])
    # chunk j partition p holds context j*128+p:
    # yck[p, j*DY+d] = y_context[j*128+p, d]
    yck = np.ascontiguousarray(
        y_context.reshape(NCHUNK, 128, DY).transpose(1, 0, 2).reshape(
            128, NCHUNK * DY
        )
    ).astype(ml_dtypes.bfloat16)
    wwt = np.ascontiguousarray(np.concatenate([W, W.T], axis=1))  # [64, 128]

    in_maps = []
    for i in range(NCORES):
        ztT = np.ascontiguousarray(
            z_target[i * TL : (i + 1) * TL].T.astype(np.float16)
        )
        m = {
            "wwt": wwt, "ztt": ztT, "yck": yck,
            "zcb0": zc0, "zcbv": zcv, "zcbt": zct,
        }
        in_maps.append(m)
    return in_maps


def postprocess(results):
    """Gather per-core [33, TL] outputs -> full (T, DY) normalized output."""
    allT = np.concatenate([r["out"].T for r in results], axis=0)  # [T, 33]
    return (allT[:, :DY] / allT[:, DY : DY + 1]).astype(np.float32)


def run(in_maps, **kwargs):
    nc = _get_nc()
    return run_bass_kernel_spmd(nc, in_maps, core_ids=list(range(NCORES)), **kwargs)


def kernel(z_context, y_context, z_target, W):
    in_maps = make_in_maps(z_context, y_context, z_target, W)
    res = run(in_maps)
    return postprocess(res.results)
